# revision 24
# baseline (speedup 1.0000x reference)
"""Trainium2 Bass kernel for nn_AdditiveMul (additive attention scores + softmax over heads).

Computes, for q:(2048,8,64) k:(2048,8,64) attn:(1,8,128) all f32:
    sq[i,h] = sum_d q[i,h,d]*attn[0,h,d]
    sk[j,h] = sum_d k[j,h,d]*attn[0,h,64+d]
    out[i,j,h] = softmax_h(relu(sq[i,h] + sk[j,h]))

Strategy (8 NeuronCores, data-parallel over q rows):
  - each core owns 256 q rows; k and attn are replicated
  - PE computes s = sq (+) sk via K=2 matmuls (lhsT rows [sqT_h; 1], rhs rows [1; skT_h])
  - ACT computes E' = exp(s) PSUM->SBUF (bf16, h-major layout)
  - DVE computes E = max(E', 1)  (= exp(relu(s)), since exp is monotone)
  - DVE adds the 8 head slices pairwise -> D = sum_h E
  - ACT computes Dinv = exp(-ln(D))   (Reciprocal activation is disallowed)
  - DVE multiplies E * Dinv writing f32 h-fast (j,h) interleaved -> contiguous DMA out
"""

import numpy as np
from contextlib import ExitStack

import concourse.bass as bass
import concourse.tile as tile
from concourse import bacc, mybir
from concourse.bass_utils import run_bass_kernel_spmd
from concourse.masks import make_identity

# Prefer the activation-function set that holds BOTH Exp and Ln so the
# per-iteration Exp...Ln...Exp sequence doesn't thrash ACT_TABLE_LOADs.
_orig_get_tables = bacc.get_activation_tables


def _tables_ln_exp_first(arch):
    t = _orig_get_tables(arch)
    pref = "natural_log_exp_and_others"
    if pref not in t:
        return t
    exp_t = mybir.ActivationFunctionType.Exp
    ln_t = mybir.ActivationFunctionType.Ln
    out = {}
    for k, v in t.items():   # keep order — positions are act_func_set_ids
        if k != pref:
            v = v - {exp_t, ln_t}
        out[k] = v
    return out


bacc.get_activation_tables = _tables_ln_exp_first

F32 = mybir.dt.float32
BF16 = mybir.dt.bfloat16
AX = mybir.AxisListType
OP = mybir.AluOpType
AF = mybir.ActivationFunctionType

N_Q, N_K, H, D = 2048, 2048, 8, 64
CORES = 8
QS = N_Q // CORES          # 256 q rows per core
NIT = QS // 128            # 2 i-tiles of 128 rows
HD = H * D                 # 512
JH = N_K * H               # 16384 output cols per q row
JC = 512                   # j-chunk (psum bank free size in f32)
NJC = N_K // JC            # 4


def _build_kernel(ctx, tc, o_d, q_d, k_d, a_d):
    nc = tc.nc

    const = ctx.enter_context(tc.tile_pool(name="const", bufs=1))

    ident = const.tile([128, 128], F32)
    make_identity(nc, ident[:])

    ones_row = const.tile([1, 128], F32)
    nc.vector.memset(ones_row[:], 1.0)

    aq_b = const.tile([128, H, D], F32)
    ak_b = const.tile([128, H, D], F32)

    # ---- per-i-tile lhsT staging: rows 2h=sqT_h, 2h+1=ones ----
    lhsT = [const.tile([2 * H, 128], BF16, name=f"lhsT{i}") for i in range(NIT)]
    # ---- rhs staging, split per j-chunk so matmuls can start before the
    # whole k preamble finishes ----
    rhs_st = [const.tile([2 * H, JC], BF16, name=f"rhs_st{g}") for g in range(NJC)]
    # ---- repacked at base partition 0 (PE requires base partition 0/32/64) ----
    lhsT_pk = const.tile([2, NIT * H, 128], BF16)
    rhs_pk = [const.tile([2, H, JC], BF16, name=f"rhs_pk{g}") for g in range(NJC)]

    pre = ctx.enter_context(tc.tile_pool(name="pre", bufs=3))
    pps = ctx.enter_context(tc.tile_pool(name="pre_ps", bufs=2, space="PSUM"))
    if True:
        # broadcast aq/ak (64 coeffs per head) to all 128 partitions via PE
        attn_sb = pre.tile([1, H * 2 * D], F32)
        nc.sync.dma_start(attn_sb[:], a_d[:])
        attn_v = attn_sb[:].rearrange("o (h w) -> o h w", h=H)   # (1, 8, 128)
        for dst, lo in ((aq_b, 0), (ak_b, D)):
            t = pps.tile([128, H * D], F32, tag="bcast", bufs=1)
            nc.tensor.matmul(t[:], ones_row[:], attn_v[:, :, lo:lo + D],
                             start=True, stop=True)
            nc.vector.tensor_copy(dst[:], t[:].rearrange("p (h w) -> p h w", h=H))

        for it in range(NIT):
            qt = pre.tile([128, HD], F32)
            nc.sync.dma_start(qt[:], q_d[it * 128:(it + 1) * 128, :])
            prod = pre.tile([128, HD], F32)
            nc.vector.tensor_tensor(prod[:], qt[:], aq_b[:].rearrange("p h w -> p (h w)"), op=OP.mult)
            so = pre.tile([128, 2 * H], F32)
            nc.vector.memset(so[:], 1.0)
            nc.vector.tensor_reduce(
                so[:].rearrange("p (x two) -> p x two", two=2)[:, :, 0:1],
                prod[:].rearrange("p (h w) -> p h w", h=H),
                axis=AX.X, op=OP.add)
            pt = pps.tile([2 * H, 128], F32)
            nc.tensor.transpose(pt[:], so[:], ident[:])
            nc.vector.tensor_copy(lhsT[it][:], pt[:])
        for it in range(NIT):
            for h in range(H):
                nc.sync.dma_start(lhsT_pk[:, it * H + h, :],
                                  lhsT[it][2 * h:2 * h + 2, :])

        for t in range(N_K // 128):
            g, tl = t // 4, t % 4
            kt = pre.tile([128, HD], F32)
            nc.sync.dma_start(kt[:], k_d[t * 128:(t + 1) * 128, :])
            prodk = pre.tile([128, HD], F32)
            # split the 16 multiplies between GpSimd and DVE so neither
            # serializes the whole preamble
            teng = nc.gpsimd if t % 2 == 0 else nc.vector
            teng.tensor_tensor(prodk[:], kt[:], ak_b[:].rearrange("p h w -> p (h w)"), op=OP.mult)
            sko = pre.tile([128, 2 * H], F32)
            nc.vector.memset(sko[:], 1.0)
            nc.vector.tensor_reduce(
                sko[:].rearrange("p (x two) -> p x two", two=2)[:, :, 1:2],
                prodk[:].rearrange("p (h w) -> p h w", h=H),
                axis=AX.X, op=OP.add)
            pt = pps.tile([2 * H, 128], F32)
            nc.tensor.transpose(pt[:], sko[:], ident[:])
            nc.vector.tensor_copy(rhs_st[g][:, tl * 128:(tl + 1) * 128], pt[:])
            if tl == 3:
                for h in range(H):
                    nc.sync.dma_start(rhs_pk[g][:, h, :],
                                      rhs_st[g][2 * h:2 * h + 2, :])

    # ---- main loop ----
    ppool = ctx.enter_context(tc.tile_pool(name="P", bufs=2, space="PSUM"))
    epool = ctx.enter_context(tc.tile_pool(name="E", bufs=2))
    dpool = ctx.enter_context(tc.tile_pool(name="dtree", bufs=4))
    rpool = ctx.enter_context(tc.tile_pool(name="recip", bufs=2))
    opool = ctx.enter_context(tc.tile_pool(name="out", bufs=2))

    for it in range(NIT):
        E = epool.tile([128, H, N_K], BF16)          # h-major
        for jc in range(NJC):
            for hg in range(4):                       # 2 heads per psum tile
                P4 = ppool.tile([128, 2, JC], F32)
                for hl in range(2):
                    h = hg * 2 + hl
                    nc.tensor.matmul(
                        P4[:, hl, :],
                        lhsT_pk[:, it * H + h, :],
                        rhs_pk[jc][:, h, :],
                        start=True, stop=True)
                nc.scalar.activation(
                    E[:, hg * 2:(hg + 1) * 2, jc * JC:(jc + 1) * JC],
                    P4[:], AF.Exp)
        # E = max(E, 1)  == exp(relu(s)); per half-head-group for overlap
        for hg in range(2):
            nc.vector.tensor_scalar(
                E[:, hg * 4:(hg + 1) * 4, :], E[:, hg * 4:(hg + 1) * 4, :],
                1.0, None, op0=OP.max)
        # D = sum_h E  (pairwise tree, bf16 2x mode)
        t01 = dpool.tile([128, N_K], BF16, tag="dt")
        t23 = dpool.tile([128, N_K], BF16, tag="dt")
        nc.vector.tensor_tensor(t01[:], E[:, 0, :], E[:, 1, :], op=OP.add)
        nc.vector.tensor_tensor(t23[:], E[:, 2, :], E[:, 3, :], op=OP.add)
        t03 = dpool.tile([128, N_K], BF16, tag="dt")
        nc.vector.tensor_tensor(t03[:], t01[:], t23[:], op=OP.add)
        t45 = dpool.tile([128, N_K], BF16, tag="dt")
        t67 = dpool.tile([128, N_K], BF16, tag="dt")
        nc.vector.tensor_tensor(t45[:], E[:, 4, :], E[:, 5, :], op=OP.add)
        nc.vector.tensor_tensor(t67[:], E[:, 6, :], E[:, 7, :], op=OP.add)
        t47 = dpool.tile([128, N_K], BF16, tag="dt")
        nc.vector.tensor_tensor(t47[:], t45[:], t67[:], op=OP.add)
        Dt = rpool.tile([128, N_K], BF16)
        nc.vector.tensor_tensor(Dt[:], t03[:], t47[:], op=OP.add)
        # Dinv = 1/D via exp(-ln(D))  (in-place Ln, then negated Exp)
        tln = rpool.tile([128, N_K], F32)
        nc.scalar.activation(tln[:], Dt[:], AF.Ln)
        Dinv = rpool.tile([128, N_K], BF16)
        nc.scalar.activation(Dinv[:], tln[:], AF.Exp, scale=-1.0)
        # out_h = E_h * Dinv: contiguous bf16 planes (2x DVE mode), one per head
        for h in range(H):
            oh = opool.tile([128, N_K], BF16)
            eng = nc.gpsimd if h < 2 else nc.vector
            eng.tensor_tensor(oh[:], E[:, h, :], Dinv[:], op=OP.mult)
            nc.sync.dma_start(o_d[h, it * 128:(it + 1) * 128, :], oh[:])


_CACHE = {}


def _get_nc():
    if "nc" not in _CACHE:
        nc = bacc.Bacc("TRN2", target_bir_lowering=False, debug=False,
                       num_devices=CORES)
        q_d = nc.dram_tensor("q", [QS, HD], F32, kind="ExternalInput").ap()
        k_d = nc.dram_tensor("k", [N_K, HD], F32, kind="ExternalInput").ap()
        a_d = nc.dram_tensor("attn", [1, H * 2 * D], F32, kind="ExternalInput").ap()
        o_d = nc.dram_tensor("out", [H, QS, N_K], BF16, kind="ExternalOutput").ap()
        with tile.TileContext(nc) as tc:
            with ExitStack() as ctx:
                _build_kernel(ctx, tc, o_d, q_d, k_d, a_d)
        nc.compile()
        _CACHE["nc"] = nc
    return _CACHE["nc"]


def run(q, k, attn, **spmd_kwargs):
    q = np.ascontiguousarray(np.asarray(q, dtype=np.float32)).reshape(N_Q, HD)
    kf = np.ascontiguousarray(np.asarray(k, dtype=np.float32)).reshape(N_K, HD)
    af = np.ascontiguousarray(np.asarray(attn, dtype=np.float32)).reshape(1, H * 2 * D)
    in_maps = [
        {"q": np.ascontiguousarray(q[c * QS:(c + 1) * QS]), "k": kf, "attn": af}
        for c in range(CORES)
    ]
    nc = _get_nc()
    res = run_bass_kernel_spmd(nc, in_maps, core_ids=list(range(CORES)),
                               **spmd_kwargs)
    full = np.empty((N_Q, N_K, H), dtype=np.float32)
    for c in range(CORES):
        shard = np.asarray(res.results[c]["out"]).astype(np.float32)  # (H, QS, N_K)
        full[c * QS:(c + 1) * QS] = shard.transpose(1, 2, 0)
    return full, res


def kernel(q, k, attn):
    return run(q, k, attn)[0]


# revision 31
# speedup vs baseline: 1.0899x; 1.0899x over previous
"""Trainium2 Bass kernel for nn_AdditiveMul (additive attention scores + softmax over heads).

Computes, for q:(2048,8,64) k:(2048,8,64) attn:(1,8,128) all f32:
    sq[i,h] = sum_d q[i,h,d]*attn[0,h,d]
    sk[j,h] = sum_d k[j,h,d]*attn[0,h,64+d]
    out[i,j,h] = softmax_h(relu(sq[i,h] + sk[j,h]))

Strategy (8 NeuronCores, data-parallel over q rows):
  - each core owns 256 q rows; k and attn are replicated
  - PE computes s = sq (+) sk via K=2 matmuls (lhsT rows [sqT_h; 1], rhs rows [1; skT_h])
  - ACT computes E' = exp(s) PSUM->SBUF (bf16, h-major layout)
  - DVE computes E = max(E', 1)  (= exp(relu(s)), since exp is monotone)
  - DVE adds the 8 head slices pairwise -> D = sum_h E
  - ACT computes Dinv = exp(-ln(D))   (Reciprocal activation is disallowed)
  - DVE multiplies E * Dinv writing f32 h-fast (j,h) interleaved -> contiguous DMA out
"""

import numpy as np
from contextlib import ExitStack

import concourse.bass as bass
import concourse.tile as tile
from concourse import bacc, mybir
from concourse.bass_utils import run_bass_kernel_spmd
from concourse.masks import make_identity

# Prefer the activation-function set that holds BOTH Exp and Ln so the
# per-iteration Exp...Ln...Exp sequence doesn't thrash ACT_TABLE_LOADs.
_orig_get_tables = bacc.get_activation_tables


def _tables_ln_exp_first(arch):
    t = _orig_get_tables(arch)
    pref = "natural_log_exp_and_others"
    if pref not in t:
        return t
    exp_t = mybir.ActivationFunctionType.Exp
    ln_t = mybir.ActivationFunctionType.Ln
    out = {}
    for k, v in t.items():   # keep order — positions are act_func_set_ids
        if k != pref:
            v = v - {exp_t, ln_t}
        out[k] = v
    return out


bacc.get_activation_tables = _tables_ln_exp_first

F32 = mybir.dt.float32
BF16 = mybir.dt.bfloat16
AX = mybir.AxisListType
OP = mybir.AluOpType
AF = mybir.ActivationFunctionType

N_Q, N_K, H, D = 2048, 2048, 8, 64
CORES = 8
QS = N_Q // CORES          # 256 q rows per core
NIT = QS // 128            # 2 i-tiles of 128 rows
HD = H * D                 # 512
JH = N_K * H               # 16384 output cols per q row
JC = 512                   # j-chunk (psum bank free size in f32)
NJC = N_K // JC            # 4


def _build_kernel(ctx, tc, o_d, q_d, k_d, a_d):
    nc = tc.nc

    const = ctx.enter_context(tc.tile_pool(name="const", bufs=1))

    ident = const.tile([128, 128], F32)
    make_identity(nc, ident[:])

    ones_row = const.tile([1, 128], F32)
    nc.vector.memset(ones_row[:], 1.0)

    aq_b = const.tile([128, H, D], F32)
    ak_b = const.tile([128, H, D], F32)

    # ---- per-i-tile lhsT staging: rows 2h=sqT_h, 2h+1=ones ----
    lhsT = [const.tile([2 * H, 128], BF16, name=f"lhsT{i}") for i in range(NIT)]
    # ---- rhs staging, split per j-chunk so matmuls can start before the
    # whole k preamble finishes ----
    rhs_st = [const.tile([2 * H, JC], BF16, name=f"rhs_st{g}") for g in range(NJC)]
    # ---- repacked at base partition 0 (PE requires base partition 0/32/64) ----
    lhsT_pk = const.tile([2, NIT * H, 128], BF16)
    rhs_pk = [const.tile([2, H, JC], BF16, name=f"rhs_pk{g}") for g in range(NJC)]

    pre = ctx.enter_context(tc.tile_pool(name="pre", bufs=2))
    pps = ctx.enter_context(tc.tile_pool(name="pre_ps", bufs=2, space="PSUM"))
    if True:
        # broadcast aq/ak (64 coeffs per head) to all 128 partitions via PE
        attn_sb = pre.tile([1, H * 2 * D], F32)
        nc.sync.dma_start(attn_sb[:], a_d[:])
        attn_v = attn_sb[:].rearrange("o (h w) -> o h w", h=H)   # (1, 8, 128)
        for dst, lo in ((aq_b, 0), (ak_b, D)):
            t = pps.tile([128, H * D], F32, tag="pt", bufs=1)
            nc.tensor.matmul(t[:], ones_row[:], attn_v[:, :, lo:lo + D],
                             start=True, stop=True)
            nc.vector.tensor_copy(dst[:], t[:].rearrange("p (h w) -> p h w", h=H))

        for it in range(NIT):
            qt = pre.tile([128, HD], F32)
            nc.sync.dma_start(qt[:], q_d[it * 128:(it + 1) * 128, :])
            prod = pre.tile([128, HD], F32)
            nc.vector.tensor_tensor(prod[:], qt[:], aq_b[:].rearrange("p h w -> p (h w)"), op=OP.mult)
            so = pre.tile([128, 2 * H], F32)
            nc.vector.memset(so[:], 1.0)
            nc.vector.tensor_reduce(
                so[:].rearrange("p (x two) -> p x two", two=2)[:, :, 0:1],
                prod[:].rearrange("p (h w) -> p h w", h=H),
                axis=AX.X, op=OP.add)
            pt = pps.tile([2 * H, 128], F32, tag="pt", bufs=1)
            nc.tensor.transpose(pt[:], so[:], ident[:])
            nc.vector.tensor_copy(lhsT[it][:], pt[:])
        for it in range(NIT):
            for h in range(H):
                nc.scalar.dma_start(lhsT_pk[:, it * H + h, :],
                                    lhsT[it][2 * h:2 * h + 2, :])

        for g in range(NJC):
            # one 1MB DMA: 512 k rows as (128 partitions, 4 tiles, 512)
            kt = pre.tile([128, 4, HD], F32)
            nc.sync.dma_start(
                kt[:], k_d[g * 512:(g + 1) * 512, :].rearrange(
                    "(t4 p) w -> p t4 w", p=128))
            prodk = pre.tile([128, 4, HD], BF16)
            teng = nc.gpsimd if g % 2 == 0 else nc.vector
            teng.tensor_tensor(
                prodk[:], kt[:],
                ak_b[:].rearrange("p h w -> p (h w)").unsqueeze(1).to_broadcast(
                    [128, 4, HD]),
                op=OP.mult)
            sko = pre.tile([128, 4, 2 * H], F32)
            nc.vector.memset(sko[:], 1.0)
            nc.vector.tensor_reduce(
                sko[:].rearrange("p t (x two) -> p t x two", two=2)[:, :, :, 1:2],
                prodk[:].rearrange("p t (h w) -> p t h w", h=H),
                axis=AX.X, op=OP.add)
            for tl in range(4):
                pt = pps.tile([2 * H, 128], F32, tag="pt", bufs=1)
                nc.tensor.transpose(pt[:], sko[:, tl, :], ident[:])
                nc.vector.tensor_copy(
                    rhs_st[g][:, tl * 128:(tl + 1) * 128], pt[:])
            for h in range(H):
                nc.scalar.dma_start(rhs_pk[g][:, h, :],
                                    rhs_st[g][2 * h:2 * h + 2, :])

    # ---- main loop ----
    ppool = ctx.enter_context(tc.tile_pool(name="P", bufs=2, space="PSUM"))
    epool = ctx.enter_context(tc.tile_pool(name="E", bufs=2))
    dpool = ctx.enter_context(tc.tile_pool(name="dtree", bufs=4))
    rpool = ctx.enter_context(tc.tile_pool(name="recip", bufs=2))
    opool = ctx.enter_context(tc.tile_pool(name="out", bufs=2))

    for it in range(NIT):
        E = epool.tile([128, H, N_K], BF16)          # h-major
        for jc in range(NJC):
            for h0, nh in ((0, 3), (3, 3), (6, 2)):   # 3+3+2 heads per psum tile
                P4 = ppool.tile([128, 3, JC], F32, tag="P")
                for hl in range(nh):
                    h = h0 + hl
                    nc.tensor.matmul(
                        P4[:, hl, :],
                        lhsT_pk[:, it * H + h, :],
                        rhs_pk[jc][:, h, :],
                        start=True, stop=True)
                nc.scalar.activation(
                    E[:, h0:h0 + nh, jc * JC:(jc + 1) * JC],
                    P4[:, 0:nh, :], AF.Exp)
        # E = max(E, 1)  == exp(relu(s)); per half-head-group for overlap
        for hg in range(2):
            nc.vector.tensor_scalar(
                E[:, hg * 4:(hg + 1) * 4, :], E[:, hg * 4:(hg + 1) * 4, :],
                1.0, None, op0=OP.max)
        # D = sum_h E  (pairwise tree, bf16 2x mode)
        t01 = dpool.tile([128, N_K], BF16, tag="dt")
        t23 = dpool.tile([128, N_K], BF16, tag="dt")
        nc.vector.tensor_tensor(t01[:], E[:, 0, :], E[:, 1, :], op=OP.add)
        nc.vector.tensor_tensor(t23[:], E[:, 2, :], E[:, 3, :], op=OP.add)
        t03 = dpool.tile([128, N_K], BF16, tag="dt")
        nc.vector.tensor_tensor(t03[:], t01[:], t23[:], op=OP.add)
        t45 = dpool.tile([128, N_K], BF16, tag="dt")
        t67 = dpool.tile([128, N_K], BF16, tag="dt")
        nc.vector.tensor_tensor(t45[:], E[:, 4, :], E[:, 5, :], op=OP.add)
        nc.vector.tensor_tensor(t67[:], E[:, 6, :], E[:, 7, :], op=OP.add)
        t47 = dpool.tile([128, N_K], BF16, tag="dt")
        nc.vector.tensor_tensor(t47[:], t45[:], t67[:], op=OP.add)
        Dt = rpool.tile([128, N_K], BF16)
        nc.vector.tensor_tensor(Dt[:], t03[:], t47[:], op=OP.add)
        # Dinv = 1/D via exp(-ln(D))  (in-place Ln, then negated Exp)
        tln = rpool.tile([128, N_K], F32, bufs=1)
        nc.scalar.activation(tln[:], Dt[:], AF.Ln)
        Dinv = rpool.tile([128, N_K], BF16)
        nc.scalar.activation(Dinv[:], tln[:], AF.Exp, scale=-1.0)
        # out_h = E_h * Dinv: contiguous bf16 planes (2x DVE mode), one per head
        for h in range(H):
            oh = opool.tile([128, N_K], BF16)
            nc.vector.tensor_tensor(oh[:], E[:, h, :], Dinv[:], op=OP.mult)
            nc.sync.dma_start(o_d[h, it * 128:(it + 1) * 128, :], oh[:])


_CACHE = {}


def _get_nc():
    if "nc" not in _CACHE:
        nc = bacc.Bacc("TRN2", target_bir_lowering=False, debug=False,
                       num_devices=CORES)
        q_d = nc.dram_tensor("q", [QS, HD], F32, kind="ExternalInput").ap()
        k_d = nc.dram_tensor("k", [N_K, HD], F32, kind="ExternalInput").ap()
        a_d = nc.dram_tensor("attn", [1, H * 2 * D], F32, kind="ExternalInput").ap()
        o_d = nc.dram_tensor("out", [H, QS, N_K], BF16, kind="ExternalOutput").ap()
        with tile.TileContext(nc) as tc:
            with ExitStack() as ctx:
                _build_kernel(ctx, tc, o_d, q_d, k_d, a_d)
        nc.compile()
        _CACHE["nc"] = nc
    return _CACHE["nc"]


def run(q, k, attn, **spmd_kwargs):
    q = np.ascontiguousarray(np.asarray(q, dtype=np.float32)).reshape(N_Q, HD)
    kf = np.ascontiguousarray(np.asarray(k, dtype=np.float32)).reshape(N_K, HD)
    af = np.ascontiguousarray(np.asarray(attn, dtype=np.float32)).reshape(1, H * 2 * D)
    in_maps = [
        {"q": np.ascontiguousarray(q[c * QS:(c + 1) * QS]), "k": kf, "attn": af}
        for c in range(CORES)
    ]
    nc = _get_nc()
    res = run_bass_kernel_spmd(nc, in_maps, core_ids=list(range(CORES)),
                               **spmd_kwargs)
    full = np.empty((N_Q, N_K, H), dtype=np.float32)
    for c in range(CORES):
        shard = np.asarray(res.results[c]["out"]).astype(np.float32)  # (H, QS, N_K)
        full[c * QS:(c + 1) * QS] = shard.transpose(1, 2, 0)
    return full, res


def kernel(q, k, attn):
    return run(q, k, attn)[0]
